# revision 1
# baseline (speedup 1.0000x reference)
"""Causal self-attention (B=4, T=2048, C=1024, NH=16) on 8 TRN2 NeuronCores.

Sharding: tensor-parallel over heads — 2 heads per core. Each core computes
its slice of qkv (transposed layout), full causal attention for its heads,
and a partial output projection; the host sums the 8 partials and adds b_proj.

Matmuls run in float32r (fp32 bits, reduced-precision PE mode, ~1.6e-4 rel
err) which streams at bf16 speed for free dims >= 256.

Layout notes:
 - qkv is computed transposed (qT/kT/vT: [dim, tok]) so scores can be formed
   as scoresT[k, q] = kT.T @ qT with d on partitions; softmax runs over the
   partition (k) axis using exp on ACT, a ones-column in the p@v matmul for
   the denominator, and a K=1 broadcast matmul for the reciprocal.
 - x is transposed on the host once (layout prep during sharding) so qkv
   needs no on-device transposes; v is re-transposed to natural layout on PE.
"""

import sys

import numpy as np

try:
    import concourse.bass as bass
except ImportError:  # grading container may not have it on sys.path
    sys.path.insert(0, "/opt/trn_rl_repo")
    import concourse.bass as bass

from contextlib import ExitStack

import concourse.mybir as mybir
import concourse.tile as tile
from concourse.bass_utils import run_bass_kernel_spmd


B, T, C, NH, HD = 4, 2048, 1024, 16, 64
N_CORES = 8
HPC = NH // N_CORES  # heads per core = 2
DPC = HPC * HD  # dims per core = 128
BT = B * T  # 8192
QCH = 512  # q-chunk (moving free dim)
KCH = 128  # k-chunk (contraction tile)
TCH = 512  # token chunk for qkv
F32 = mybir.dt.float32
F32R = mybir.dt.float32r
AF = mybir.ActivationFunctionType


def _r(ap):
    return ap.bitcast(F32R)


def _act_reciprocal(nc, out, in_):
    """Reciprocal on the scalar engine (~430ns for [1,512] vs ~3.3us for
    nc.vector.reciprocal's Newton chain). bass blocks AF.Reciprocal behind an
    accuracy warning; the spline is good to ~1e-5 rel which is far below this
    kernel's f32r noise floor, so emit the instruction directly."""
    eng = nc.scalar
    ins = [
        eng.lower_ap(in_),
        mybir.ImmediateValue(dtype=mybir.dt.float32, value=0.0),
        mybir.ImmediateValue(dtype=mybir.dt.float32, value=1.0),
        mybir.ImmediateValue(dtype=mybir.dt.float32, value=0.0),
    ]
    return eng.add_instruction(
        mybir.InstActivation(
            name=nc.get_next_instruction_name(),
            func=AF.Reciprocal,
            ins=ins,
            outs=[eng.lower_ap(out)],
        )
    )


def _split_multi_waits(nc):
    """Walrus in this container accepts only ONE sync wait per instruction.
    Hoist extra waits onto same-engine NoOps inserted just before."""
    n = 0
    for f in nc.m.functions:
        for b in f.blocks:
            insts = b.instructions
            if not any(
                i.sync_info is not None
                and i.sync_info.on_wait
                and len(i.sync_info.on_wait) > 1
                for i in insts
            ):
                continue
            new = []
            for ins in insts:
                si = ins.sync_info
                if si is not None and si.on_wait and len(si.on_wait) > 1:
                    waits = list(si.on_wait)
                    for w in waits[:-1]:
                        nop = mybir.InstNoOp(
                            name=f"{ins.name}-ws{n}", ins=[], outs=[]
                        )
                        nop.engine = ins.engine
                        nop.bass_nofuse = True
                        nop.sync_info = mybir.SyncInfo(on_wait=[w], on_update=[])
                        if ins.debug is not None:
                            nop.debug = ins.debug
                        new.append(nop)
                        n += 1
                    ins.sync_info = mybir.SyncInfo(
                        on_wait=[waits[-1]], on_update=list(si.on_update or [])
                    )
                new.append(ins)
            b.instructions = new
    return n


def build_kernel():
    nc = bass.Bass("TRN2", target_bir_lowering=False, debug=False, num_devices=N_CORES)
    xT_d = nc.dram_tensor("xT", [C, BT], F32R, kind="ExternalInput")
    wc_d = nc.dram_tensor("wc", [C, 3 * DPC], F32R, kind="ExternalInput")
    bc_d = nc.dram_tensor("bc", [3, DPC, 1], F32, kind="ExternalInput")
    wp_d = nc.dram_tensor("wp", [DPC, C], F32R, kind="ExternalInput")
    out_d = nc.dram_tensor("out", [BT, C], F32, kind="ExternalOutput")

    with tile.TileContext(nc) as tc, ExitStack() as ctx:
        consts = ctx.enter_context(tc.tile_pool(name="consts", bufs=1))
        xpool = ctx.enter_context(tc.tile_pool(name="x", bufs=16))
        qkvp = ctx.enter_context(tc.tile_pool(name="qkv", bufs=2))
        vexp = ctx.enter_context(tc.tile_pool(name="vext", bufs=2))
        ytp = ctx.enter_context(tc.tile_pool(name="yt", bufs=2))
        expp = ctx.enter_context(tc.tile_pool(name="expt", bufs=10))
        smallp = ctx.enter_context(tc.tile_pool(name="small", bufs=2))
        outp = ctx.enter_context(tc.tile_pool(name="outt", bufs=4))
        ps_acc = ctx.enter_context(tc.tile_pool(name="ps_acc", bufs=5, space="PSUM"))
        ps_sc = ctx.enter_context(tc.tile_pool(name="ps_sc", bufs=3, space="PSUM"))

        # [128, 64] tile holding I64 in partitions 0-63 and again in 64-127,
        # so each head's vT slice has an identity at its own base partition.
        ident = consts.tile([128, 64], F32)
        nc.gpsimd.memset(ident, 0.0)
        for half in range(2):
            nc.gpsimd.affine_select(
                out=ident[64 * half : 64 * half + 64, :],
                in_=ident[64 * half : 64 * half + 64, :],
                compare_op=mybir.AluOpType.not_equal,
                fill=1.0,
                base=0,
                pattern=[[-1, 64]],
                channel_multiplier=1,
            )
        ones_row = consts.tile([1, 64], F32R)
        nc.vector.memset(ones_row.bitcast(F32), 1.0)

        # weights: wc [1024, 384] -> [128, 8, 384] (kc chunks on free dim)
        w_sb = consts.tile([128, 8, 3 * DPC], F32R)
        nc.sync.dma_start(
            out=w_sb, in_=wc_d.ap().rearrange("(kc p) c -> p kc c", p=128)
        )
        wp_sb = consts.tile([128, C], F32R)
        nc.sync.dma_start(out=wp_sb, in_=wp_d.ap())
        bc_sb = consts.tile([128, 3], F32)
        nc.sync.dma_start(out=bc_sb, in_=bc_d.ap().rearrange("g p one -> p (g one)"))

        NKC = C // 128  # 8 contraction chunks for qkv
        NTC = T // TCH  # 4 token chunks per batch
        NQC = T // QCH  # 4 q-chunks per batch (per head)
        NVC = T // 128  # 16 v chunks per batch

        # -------- unit-based emission with explicit cross-phase interleave.
        # Each "unit" is a thunk emitting a small group of instructions.
        # Attention(b) is ACT-bound (exp), so qkv(b+1), vT(b+1) and proj(b-1)
        # units are spliced between its j-iterations to keep PE dense.
        state = {}

        def qkv_units(b):
            t0 = b * T
            st = state.setdefault(b, {})
            units = []

            def alloc(b=b, st=st):
                st["qT"] = qkvp.tile([128, T], F32R, name=f"qT_{b}", tag="qT")
                st["kT"] = qkvp.tile([128, T], F32R, name=f"kT_{b}", tag="kT")
                st["vT"] = qkvp.tile([128, T], F32, name=f"vT_{b}", tag="vT")
                st["xts"] = {}

            units.append(alloc)
            for tcb in range(NTC):

                def dma_u(tcb=tcb, st=st, t0=t0):
                    xts = []
                    for kc in range(NKC):
                        xt = xpool.tile(
                            [128, TCH], F32R, name=f"xt_{b}_{tcb}_{kc}", tag="xt"
                        )
                        nc.sync.dma_start(
                            out=xt,
                            in_=xT_d.ap()[
                                kc * 128 : (kc + 1) * 128,
                                t0 + tcb * TCH : t0 + (tcb + 1) * TCH,
                            ],
                        )
                        xts.append(xt)
                    st["xts"][tcb] = xts

                units.append(dma_u)
                for g in range(3):

                    def mm_u(tcb=tcb, g=g, st=st):
                        dest = [st["qT"], st["kT"], st["vT"]]
                        ps = ps_acc.tile(
                            [128, TCH], F32, name=f"qkvps_{b}_{tcb}_{g}", tag="acc"
                        )
                        for kc in range(NKC):
                            nc.tensor.matmul(
                                ps,
                                w_sb[:, kc, g * 128 : (g + 1) * 128],
                                st["xts"][tcb][kc],
                                start=(kc == 0),
                                stop=(kc == NKC - 1),
                            )
                        # psum -> sbuf with bias add, on DVE (ACT stays on exp)
                        nc.vector.tensor_scalar_add(
                            dest[g][:, tcb * TCH : (tcb + 1) * TCH],
                            ps,
                            bc_sb[:, g : g + 1],
                        )

                    units.append(mm_u)
            return units

        def vt_units(b):
            st = state.setdefault(b, {})
            units = []

            def alloc(st=st, b=b):
                st["vex"] = vexp.tile(
                    [128, HPC, NVC, 65], F32R, name=f"vex_{b}", tag="vex"
                )
                nc.vector.memset(st["vex"][:, :, :, 64:65].bitcast(F32), 1.0)

            units.append(alloc)
            for h in range(HPC):
                for j0 in range(0, NVC, 4):

                    def tr_u(h=h, j0=j0, st=st):
                        for j in range(j0, j0 + 4):
                            pt = ps_sc.tile(
                                [128, 64], F32, name=f"vtps_{b}_{h}_{j}", tag="sc"
                            )
                            nc.tensor.transpose(
                                pt,
                                st["vT"][64 * h : 64 * h + 64, j * 128 : (j + 1) * 128],
                                ident[64 * h : 64 * h + 64, :],
                            )
                            nc.vector.tensor_copy(st["vex"][:, h, j, 0:64], pt)

                    units.append(tr_u)
            return units

        def attn_units(b):
            st = state[b]
            units = []

            def alloc_yt(st=st, b=b):
                st["yT"] = ytp.tile([128, T], F32R, name=f"yT_{b}", tag="yT")

            units.append(alloc_yt)
            for h in range(HPC):

                def alloc_chains(h=h, st=st, b=b):
                    st[("yts", h)] = [
                        ps_acc.tile(
                            [65, QCH], F32, name=f"yt_{b}_{h}_{qc}", tag="acc"
                        )
                        for qc in range(NQC)
                    ]

                units.append(alloc_chains)
                for j in range(NVC):

                    def j_u(h=h, j=j, st=st, b=b):
                        qTh = st["qT"][64 * h : 64 * h + 64, :]
                        kTh = st["kT"][64 * h : 64 * h + 64, :]
                        k0 = j * KCH
                        exs = []
                        for qc in range(NQC):
                            q0 = qc * QCH
                            if k0 >= q0 + QCH:
                                continue
                            qlo = max(0, k0 - q0)
                            sc = ps_sc.tile(
                                [128, QCH], F32, name=f"sc_{b}_{h}_{j}_{qc}", tag="sc"
                            )
                            nc.tensor.matmul(
                                sc[:, qlo:QCH],
                                kTh[:, k0 : k0 + KCH],
                                qTh[:, q0 + qlo : q0 + QCH],
                                start=True,
                                stop=True,
                            )
                            ex = expp.tile(
                                [128, QCH], F32R, name=f"ex_{b}_{h}_{j}_{qc}", tag="ex"
                            )
                            nc.scalar.activation(
                                ex[:, qlo:QCH], sc[:, qlo:QCH], AF.Exp, scale=0.125
                            )
                            if k0 >= q0:
                                # diagonal 128-wide block: zero where k > q
                                nc.gpsimd.affine_select(
                                    out=ex[:, qlo : qlo + 128],
                                    in_=ex[:, qlo : qlo + 128],
                                    compare_op=mybir.AluOpType.is_ge,
                                    fill=0.0,
                                    base=0,
                                    pattern=[[1, 128]],
                                    channel_multiplier=-1,
                                )
                            exs.append((qc, ex, qlo))
                        for qc, ex, qlo in exs:
                            q0 = qc * QCH
                            nk = (q0 + QCH) // KCH
                            nc.tensor.matmul(
                                st[("yts", h)][qc][:, qlo:QCH],
                                st["vex"][:, h, j, :],
                                ex[:, qlo:QCH],
                                start=(j == 0),
                                stop=(j == nk - 1),
                            )

                    units.append(j_u)

                def norm_u(h=h, st=st, b=b):
                    # gather the 4 q-chunks' sums, then one Ln and one Exp
                    # over [1, 4*QCH]: 1/s = exp(-ln(s)) on ACT (same table
                    # set as Exp; AF.Reciprocal forces a ~1.3us table swap)
                    sums4 = smallp.tile(
                        [1, NQC * QCH], F32, name=f"sums_{b}_{h}", tag="lns"
                    )
                    for qc in range(NQC):
                        nc.vector.tensor_copy(
                            sums4[:, qc * QCH : (qc + 1) * QCH],
                            st[("yts", h)][qc][64:65, :],
                        )
                    recip4 = smallp.tile(
                        [1, NQC * QCH], F32R, name=f"rc_{b}_{h}", tag="recip"
                    )
                    nc.scalar.activation(
                        sums4, sums4, AF.Ln
                    )
                    nc.scalar.activation(recip4, sums4, AF.Exp, scale=-1.0)
                    for qc in range(NQC):
                        yt_ps = st[("yts", h)][qc]
                        q0 = qc * QCH
                        bc_ps = ps_sc.tile(
                            [64, QCH], F32, name=f"bc_{b}_{h}_{qc}", tag="sc"
                        )
                        nc.tensor.matmul(
                            bc_ps,
                            ones_row,
                            recip4[:, qc * QCH : (qc + 1) * QCH],
                            start=True,
                            stop=True,
                        )
                        bc_sb2 = smallp.tile(
                            [64, QCH], F32, name=f"bcs_{b}_{h}_{qc}", tag="bcast"
                        )
                        nc.vector.tensor_copy(bc_sb2, bc_ps)
                        nc.vector.tensor_mul(
                            st["yT"][64 * h : 64 * h + 64, q0 : q0 + QCH],
                            yt_ps[0:64, :],
                            bc_sb2,
                        )

                units.append(norm_u)
            return units

        def proj_units(b):
            st = state[b]
            t0 = b * T
            units = []
            for tcb in range(T // 128):

                def p_u(tcb=tcb, st=st, t0=t0, b=b):
                    for g in range(2):
                        ps = ps_acc.tile(
                            [128, 512], F32, name=f"pps_{b}_{tcb}_{g}", tag="acc"
                        )
                        nc.tensor.matmul(
                            ps,
                            st["yT"][:, tcb * 128 : (tcb + 1) * 128],
                            wp_sb[:, g * 512 : (g + 1) * 512],
                            start=True,
                            stop=True,
                        )
                        ot = outp.tile(
                            [128, 512], F32, name=f"ot_{b}_{tcb}_{g}", tag="ot"
                        )
                        nc.vector.tensor_copy(ot, ps)
                        nc.sync.dma_start(
                            out=out_d.ap()[
                                t0 + tcb * 128 : t0 + (tcb + 1) * 128,
                                g * 512 : (g + 1) * 512,
                            ],
                            in_=ot,
                        )

                units.append(p_u)
            return units

        def interleave(main, fill):
            """emit main units with fill units spread evenly between them"""
            out = []
            nf, nm = len(fill), len(main)
            fi = 0
            for mi, m in enumerate(main):
                out.append(m)
                want = (mi + 1) * nf // nm
                while fi < want:
                    out.append(fill[fi])
                    fi += 1
            out.extend(fill[fi:])
            return out

        for u in qkv_units(0) + vt_units(0):
            u()
        for b in range(B):
            main = attn_units(b)
            fill = []
            if b + 1 < B:
                fill += qkv_units(b + 1) + vt_units(b + 1)
            if b >= 1:
                fill += proj_units(b - 1)
            for u in interleave(main, fill):
                u()
        for u in proj_units(B - 1):
            u()

    _split_multi_waits(nc)
    return nc


_NC_CACHE = None


def _get_nc():
    global _NC_CACHE
    if _NC_CACHE is None:
        _NC_CACHE = build_kernel()
    return _NC_CACHE


def kernel_with_results(x, W_attn, b_attn, W_proj, b_proj, trace=False):
    x = np.asarray(x, dtype=np.float32)
    W_attn = np.asarray(W_attn, dtype=np.float32)
    b_attn = np.asarray(b_attn, dtype=np.float32)
    W_proj = np.asarray(W_proj, dtype=np.float32)
    b_proj = np.asarray(b_proj, dtype=np.float32)

    xT = np.ascontiguousarray(x.reshape(BT, C).T)  # [C, BT]
    in_maps = []
    for c in range(N_CORES):
        lo = c * DPC
        wc = np.ascontiguousarray(
            np.concatenate(
                [
                    W_attn[:, lo : lo + DPC],
                    W_attn[:, C + lo : C + lo + DPC],
                    W_attn[:, 2 * C + lo : 2 * C + lo + DPC],
                ],
                axis=1,
            )
        )
        bc = np.ascontiguousarray(
            np.stack(
                [
                    b_attn[lo : lo + DPC],
                    b_attn[C + lo : C + lo + DPC],
                    b_attn[2 * C + lo : 2 * C + lo + DPC],
                ]
            ).reshape(3, DPC, 1)
        )
        wp = np.ascontiguousarray(W_proj[lo : lo + DPC, :])
        in_maps.append({"xT": xT, "wc": wc, "bc": bc, "wp": wp})

    nc = _get_nc()
    res = run_bass_kernel_spmd(
        nc, in_maps, core_ids=list(range(N_CORES)), trace=trace
    )
    acc = np.zeros((BT, C), dtype=np.float64)
    for c in range(N_CORES):
        acc += res.results[c]["out"].astype(np.float64)
    out = (acc + b_proj.astype(np.float64)).astype(np.float32)
    return out.reshape(B, T, C), res


def kernel(x, W_attn, b_attn, W_proj, b_proj):
    out, _ = kernel_with_results(x, W_attn, b_attn, W_proj, b_proj)
    return out



# revision 9
# speedup vs baseline: 1.5917x; 1.5917x over previous
"""Causal self-attention (B=4, T=2048, C=1024, NH=16) on 8 TRN2 NeuronCores.

Sharding: tensor-parallel over heads - 2 heads per core. Each core computes
its slice of qkv (transposed layout), full causal attention for its heads,
and a partial output projection; the host sums the 8 bf16 partials and adds
b_proj.

All matmul operands are bf16 (fp32 PSUM accumulation). bf16 enables the
fast-weight-load path so LDWEIGHTS overlaps the matmul stream, and halves
DMA + DVE-copy traffic. Measured rel err stays ~1e-3, far under the 2e-2
budget.

Attention is emitted qc-major (one 512-wide q chunk at a time per batch):
for each k-chunk j the two heads' score matmuls write one [128,1024] PSUM
pair, a single fused Exp covers both heads, and pv accumulates into one
[65,512] PSUM bank per head (ones-column computes the softmax denominator).
The j-loop is software-pipelined as sc(j) / exp(j) / pv(j-1) so the PE
never sits on the ACT dependency, with qkv(b+1) / vT-transpose / proj(b-1)
units interleaved between attention units to keep the PE array dense (and
HAM-warm).

PSUM budget (8 banks): sc pair 2 + yts 2 + qkv fill 2 + proj 1 + vT 0.25.
"""

import sys

import numpy as np

try:
    import concourse.bass as bass
except ImportError:  # grading container may not have it on sys.path
    sys.path.insert(0, "/opt/trn_rl_repo")
    import concourse.bass as bass

from contextlib import ExitStack

import ml_dtypes
import concourse.mybir as mybir
import concourse.tile as tile
from concourse.bass_utils import run_bass_kernel_spmd


B, T, C, NH, HD = 4, 2048, 1024, 16, 64
N_CORES = 8
HPC = NH // N_CORES  # heads per core = 2
DPC = HPC * HD  # dims per core = 128
BT = B * T  # 8192
QCH = 512  # q-chunk
KCH = 128  # k-chunk
TCH = 512  # token chunk for qkv
NKC = C // 128  # 8 contraction chunks for qkv
NTC = T // TCH  # 4 token chunks per batch
NQC = T // QCH  # 4 q-chunks per batch
NVC = T // KCH  # 16 k/v chunks per batch
F32 = mybir.dt.float32
BF16 = mybir.dt.bfloat16
AF = mybir.ActivationFunctionType


def _split_multi_waits(nc):
    """Walrus in this container accepts only ONE sync wait per instruction.
    Hoist extra waits onto same-engine NoOps inserted just before."""
    n = 0
    for f in nc.m.functions:
        for b in f.blocks:
            insts = b.instructions
            if not any(
                i.sync_info is not None
                and i.sync_info.on_wait
                and len(i.sync_info.on_wait) > 1
                for i in insts
            ):
                continue
            new = []
            for ins in insts:
                si = ins.sync_info
                if si is not None and si.on_wait and len(si.on_wait) > 1:
                    waits = list(si.on_wait)
                    for w in waits[:-1]:
                        nop = mybir.InstNoOp(
                            name=f"{ins.name}-ws{n}", ins=[], outs=[]
                        )
                        nop.engine = ins.engine
                        nop.bass_nofuse = True
                        nop.sync_info = mybir.SyncInfo(on_wait=[w], on_update=[])
                        if ins.debug is not None:
                            nop.debug = ins.debug
                        new.append(nop)
                        n += 1
                    ins.sync_info = mybir.SyncInfo(
                        on_wait=[waits[-1]], on_update=list(si.on_update or [])
                    )
                new.append(ins)
            b.instructions = new
    return n


def build_kernel():
    nc = bass.Bass("TRN2", target_bir_lowering=False, debug=False, num_devices=N_CORES)
    xT_d = nc.dram_tensor("xT", [C, BT], BF16, kind="ExternalInput")
    wc_d = nc.dram_tensor("wc", [C, 3 * DPC], BF16, kind="ExternalInput")
    bc_d = nc.dram_tensor("bc", [3, DPC, 1], F32, kind="ExternalInput")
    wp_d = nc.dram_tensor("wp", [DPC, C], BF16, kind="ExternalInput")
    out_d = nc.dram_tensor("out", [BT, C], BF16, kind="ExternalOutput")

    with tile.TileContext(nc) as tc, ExitStack() as ctx:
        consts = ctx.enter_context(tc.tile_pool(name="consts", bufs=1))
        xpool = ctx.enter_context(tc.tile_pool(name="x", bufs=16))
        qkvp = ctx.enter_context(tc.tile_pool(name="qkv", bufs=2))
        vexp = ctx.enter_context(tc.tile_pool(name="vext", bufs=2))
        ytp = ctx.enter_context(tc.tile_pool(name="yt", bufs=2))
        expp = ctx.enter_context(tc.tile_pool(name="expt", bufs=6))
        smallp = ctx.enter_context(tc.tile_pool(name="small", bufs=3))
        outp = ctx.enter_context(tc.tile_pool(name="outt", bufs=4))
        # PSUM: 8 banks total. sc 2 + yt 2 + qk 2 + po 1 + pt 0.25
        ps_sc = ctx.enter_context(tc.tile_pool(name="ps_sc", bufs=1, space="PSUM"))
        ps_yt = ctx.enter_context(tc.tile_pool(name="ps_yt", bufs=2, space="PSUM"))
        ps_qk = ctx.enter_context(tc.tile_pool(name="ps_qk", bufs=2, space="PSUM"))
        ps_po = ctx.enter_context(tc.tile_pool(name="ps_po", bufs=1, space="PSUM"))
        ps_pt = ctx.enter_context(tc.tile_pool(name="ps_pt", bufs=1, space="PSUM"))

        ones_row = consts.tile([1, 64], BF16)
        nc.vector.memset(ones_row, 1.0)

        # [128,128] bf16 identity for PE transposes
        ident = consts.tile([128, 128], BF16)
        nc.gpsimd.memset(ident, 0.0)
        nc.gpsimd.affine_select(
            out=ident,
            in_=ident,
            compare_op=mybir.AluOpType.not_equal,
            fill=1.0,
            base=0,
            pattern=[[-1, 128]],
            channel_multiplier=1,
        )

        # weights: wc [1024, 384] -> [128, 8, 384] (kc chunks on free dim)
        w_sb = consts.tile([128, 8, 3 * DPC], BF16)
        nc.sync.dma_start(
            out=w_sb, in_=wc_d.ap().rearrange("(kc p) c -> p kc c", p=128)
        )
        wp_sb = consts.tile([128, C], BF16)
        nc.sync.dma_start(out=wp_sb, in_=wp_d.ap())
        bc_sb = consts.tile([128, 3], F32)
        nc.sync.dma_start(out=bc_sb, in_=bc_d.ap().rearrange("g p one -> p (g one)"))

        # -------- unit-based emission with explicit cross-phase interleave.
        state = {}

        def qkv_units(b):
            t0 = b * T
            st = state.setdefault(b, {})
            units = []

            def alloc(b=b, st=st):
                st["qT"] = qkvp.tile([128, T], BF16, name=f"qT_{b}", tag="qT")
                st["kT"] = qkvp.tile([128, T], BF16, name=f"kT_{b}", tag="kT")
                st["vT"] = qkvp.tile([128, T], BF16, name=f"vT_{b}", tag="vT")
                st["xts"] = {}

            units.append(alloc)
            for tcb in range(NTC):

                def dma_u(tcb=tcb, st=st, t0=t0, b=b):
                    xts = []
                    for kc in range(NKC):
                        xt = xpool.tile(
                            [128, TCH], BF16, name=f"xt_{b}_{tcb}_{kc}", tag="xt"
                        )
                        nc.sync.dma_start(
                            out=xt,
                            in_=xT_d.ap()[
                                kc * 128 : (kc + 1) * 128,
                                t0 + tcb * TCH : t0 + (tcb + 1) * TCH,
                            ],
                        )
                        xts.append(xt)
                    st["xts"][tcb] = xts

                units.append(dma_u)
                for g in range(3):
                    # split each accumulation group in two emission units so
                    # fills interleave at finer grain (same psum tile).
                    def mm_u1(tcb=tcb, g=g, st=st, b=b):
                        ps = ps_qk.tile(
                            [128, TCH], F32, name=f"qkvps_{b}_{tcb}_{g}", tag="qk"
                        )
                        st["qkv_ps"] = ps
                        for kc in range(4):
                            nc.tensor.matmul(
                                ps,
                                w_sb[:, kc, g * 128 : (g + 1) * 128],
                                st["xts"][tcb][kc],
                                start=(kc == 0),
                                stop=False,
                            )

                    def mm_u2(tcb=tcb, g=g, st=st, b=b):
                        ps = st["qkv_ps"]
                        dest = [st["qT"], st["kT"], st["vT"]]
                        for kc in range(4, NKC):
                            nc.tensor.matmul(
                                ps,
                                w_sb[:, kc, g * 128 : (g + 1) * 128],
                                st["xts"][tcb][kc],
                                start=False,
                                stop=(kc == NKC - 1),
                            )
                        # psum -> sbuf bf16 with bias add on DVE
                        nc.vector.tensor_scalar_add(
                            dest[g][:, tcb * TCH : (tcb + 1) * TCH],
                            ps,
                            bc_sb[:, g : g + 1],
                        )

                    units.append(mm_u1)
                    units.append(mm_u2)
            return units

        def vt_units(b):
            st = state.setdefault(b, {})
            units = []

            def alloc(st=st, b=b):
                st["vex"] = vexp.tile(
                    [128, NVC, HPC, 65], BF16, name=f"vex_{b}", tag="vex"
                )
                nc.vector.memset(st["vex"][:, :, :, 64:65], 1.0)

            units.append(alloc)
            for j in range(NVC):

                def tr_u(j=j, st=st, b=b):
                    # transpose both heads' [128d, 128t] chunk in one shot
                    ptf = ps_pt.tile([128, 512], F32, name=f"vtps_{b}_{j}", tag="pt")
                    pt = ptf[:, 0:64].bitcast(BF16)  # [128, 128] bf16 view
                    nc.tensor.transpose(
                        pt, st["vT"][:, j * 128 : (j + 1) * 128], ident
                    )
                    for h in range(HPC):
                        nc.vector.tensor_copy(
                            st["vex"][:, j, h, 0:64], pt[:, 64 * h : 64 * h + 64]
                        )

                units.append(tr_u)
            return units

        def attn_units(b):
            st = state[b]
            units = []

            def alloc_yt(st=st, b=b):
                st["yT"] = ytp.tile([128, T], BF16, name=f"yT_{b}", tag="yT")

            units.append(alloc_yt)
            for qc in range(NQC):
                nj = 4 * (qc + 1)  # k-chunks for this q chunk

                def alloc_chains(qc=qc, st=st, b=b):
                    st["yts"] = [
                        ps_yt.tile(
                            [65, QCH], F32, name=f"yt_{b}_{qc}_{h}", tag="yt"
                        )
                        for h in range(HPC)
                    ]

                units.append(alloc_chains)

                def sc_u(j, qc=qc, st=st, b=b):
                    q0 = qc * QCH
                    k0 = j * KCH
                    qlo = max(0, k0 - q0)
                    sc = ps_sc.tile(
                        [128, 2 * QCH], F32, name=f"sc_{b}_{qc}_{j}", tag="sc"
                    )
                    st[("sc", j)] = (sc, qlo)
                    for h in range(HPC):
                        nc.tensor.matmul(
                            sc[:, h * QCH + qlo : (h + 1) * QCH],
                            st["kT"][64 * h : 64 * h + 64, k0 : k0 + KCH],
                            st["qT"][64 * h : 64 * h + 64, q0 + qlo : q0 + QCH],
                            start=True,
                            stop=True,
                        )

                def exp_u(j, qc=qc, st=st, b=b):
                    sc, qlo = st[("sc", j)]
                    ex = expp.tile(
                        [128, 2 * QCH], BF16, name=f"ex_{b}_{qc}_{j}", tag="ex"
                    )
                    st[("ex", j)] = (ex, qlo)
                    # one fused exp across both heads (garbage cols between
                    # h0's end and h1's qlo are never consumed by pv)
                    nc.scalar.activation(
                        ex[:, qlo : 2 * QCH], sc[:, qlo : 2 * QCH], AF.Exp,
                        scale=0.125,
                    )
                    if j * KCH >= qc * QCH:
                        # diagonal block: zero where k > q, per head
                        for h in range(HPC):
                            nc.gpsimd.affine_select(
                                out=ex[:, h * QCH + qlo : h * QCH + qlo + 128],
                                in_=ex[:, h * QCH + qlo : h * QCH + qlo + 128],
                                compare_op=mybir.AluOpType.is_ge,
                                fill=0.0,
                                base=0,
                                pattern=[[1, 128]],
                                channel_multiplier=-1,
                            )

                def pv_u(j, qc=qc, nj=nj, st=st, b=b):
                    ex, qlo = st[("ex", j)]
                    for h in range(HPC):
                        nc.tensor.matmul(
                            st["yts"][h][:, qlo:QCH],
                            st["vex"][:, j, h, :],
                            ex[:, h * QCH + qlo : (h + 1) * QCH],
                            start=(j == 0),
                            stop=(j == nj - 1),
                        )

                # software pipeline: sc(j), exp(j), pv(j-1)
                for j in range(nj):
                    units.append(lambda j=j, f=sc_u: f(j))
                    units.append(lambda j=j, f=exp_u: f(j))
                    if j > 0:
                        units.append(lambda j=j, f=pv_u: f(j - 1))
                units.append(lambda nj=nj, f=pv_u: f(nj - 1))

                def norm_u1(qc=qc, st=st, b=b):
                    # 1/denominator = exp(-ln(s)); Ln+Exp share one ACT
                    # table set, and reading sums via ACT-Ln from PSUM
                    # skips a DVE gather.
                    sums = smallp.tile([1, 2 * QCH], F32, name=f"s_{b}_{qc}", tag="s")
                    st["sums"] = sums
                    for h in range(HPC):
                        nc.scalar.activation(
                            sums[:, h * QCH : (h + 1) * QCH],
                            st["yts"][h][64:65, :],
                            AF.Ln,
                        )

                def norm_u2(qc=qc, st=st, b=b):
                    recip = smallp.tile(
                        [1, 2 * QCH], BF16, name=f"rc_{b}_{qc}", tag="rc"
                    )
                    nc.scalar.activation(recip, st["sums"], AF.Exp, scale=-1.0)
                    st["recip"] = recip

                def norm_u3(h, qc=qc, st=st, b=b):
                    # broadcast recip row to 64 partitions via K=1 matmul
                    # (walrus here rejects the gpsimd partition_broadcast op)
                    bct = ps_pt.tile(
                        [128, QCH], F32, name=f"bcp_{b}_{qc}_{h}", tag="pt"
                    )
                    nc.tensor.matmul(
                        bct[0:64, :],
                        ones_row,
                        st["recip"][0:1, h * QCH : (h + 1) * QCH],
                        start=True,
                        stop=True,
                    )
                    bch = smallp.tile(
                        [64, QCH], BF16, name=f"bc_{b}_{qc}_{h}", tag=f"bc{h}"
                    )
                    nc.vector.tensor_copy(bch, bct[0:64, :])
                    nc.vector.tensor_mul(
                        st["yT"][64 * h : 64 * h + 64, qc * QCH : (qc + 1) * QCH],
                        st["yts"][h][0:64, :],
                        bch,
                    )

                units.append(norm_u1)
                units.append(norm_u2)
                units.append(lambda f=norm_u3: f(0))
                units.append(lambda f=norm_u3: f(1))
            return units

        def proj_units(b, extra_pool=False):
            st = state[b]
            t0 = b * T
            units = []
            for tcb in range(T // 128):
                for g in range(2):

                    def p_u(tcb=tcb, g=g, st=st, t0=t0, b=b):
                        pool = ps_qk if (extra_pool and g == 1) else ps_po
                        tag = "qk" if (extra_pool and g == 1) else "po"
                        ps = pool.tile(
                            [128, 512], F32, name=f"pps_{b}_{tcb}_{g}", tag=tag
                        )
                        nc.tensor.matmul(
                            ps,
                            st["yT"][:, tcb * 128 : (tcb + 1) * 128],
                            wp_sb[:, g * 512 : (g + 1) * 512],
                            start=True,
                            stop=True,
                        )
                        ot = outp.tile(
                            [128, 512], BF16, name=f"ot_{b}_{tcb}_{g}", tag="ot"
                        )
                        nc.vector.tensor_copy(ot, ps)
                        nc.sync.dma_start(
                            out=out_d.ap()[
                                t0 + tcb * 128 : t0 + (tcb + 1) * 128,
                                g * 512 : (g + 1) * 512,
                            ],
                            in_=ot,
                        )

                    units.append(p_u)
            return units

        def interleave(main, fill):
            """emit main units with fill units spread evenly between them"""
            out = []
            nf, nm = len(fill), len(main)
            fi = 0
            for mi, m in enumerate(main):
                out.append(m)
                want = (mi + 1) * nf // nm
                while fi < want:
                    out.append(fill[fi])
                    fi += 1
            out.extend(fill[fi:])
            return out

        for u in qkv_units(0) + vt_units(0):
            u()
        for b in range(B):
            main = attn_units(b)
            fill = []
            if b + 1 < B:
                fill += qkv_units(b + 1) + vt_units(b + 1)
            if b >= 1:
                fill += proj_units(b - 1, extra_pool=(b - 1 == 2))
            for u in interleave(main, fill):
                u()
        for u in proj_units(B - 1, extra_pool=True):
            u()

    _split_multi_waits(nc)
    return nc


_NC_CACHE = None


def _get_nc():
    global _NC_CACHE
    if _NC_CACHE is None:
        _NC_CACHE = build_kernel()
    return _NC_CACHE


def kernel_with_results(x, W_attn, b_attn, W_proj, b_proj, trace=False):
    bf = ml_dtypes.bfloat16
    x = np.asarray(x, dtype=np.float32)
    W_attn = np.asarray(W_attn, dtype=np.float32)
    b_attn = np.asarray(b_attn, dtype=np.float32)
    W_proj = np.asarray(W_proj, dtype=np.float32)
    b_proj = np.asarray(b_proj, dtype=np.float32)

    xT = np.ascontiguousarray(x.reshape(BT, C).T).astype(bf)  # [C, BT]
    in_maps = []
    for c in range(N_CORES):
        lo = c * DPC
        wc = np.ascontiguousarray(
            np.concatenate(
                [
                    W_attn[:, lo : lo + DPC],
                    W_attn[:, C + lo : C + lo + DPC],
                    W_attn[:, 2 * C + lo : 2 * C + lo + DPC],
                ],
                axis=1,
            )
        ).astype(bf)
        bc = np.ascontiguousarray(
            np.stack(
                [
                    b_attn[lo : lo + DPC],
                    b_attn[C + lo : C + lo + DPC],
                    b_attn[2 * C + lo : 2 * C + lo + DPC],
                ]
            ).reshape(3, DPC, 1)
        )
        wp = np.ascontiguousarray(W_proj[lo : lo + DPC, :]).astype(bf)
        in_maps.append({"xT": xT, "wc": wc, "bc": bc, "wp": wp})

    nc = _get_nc()
    res = run_bass_kernel_spmd(
        nc, in_maps, core_ids=list(range(N_CORES)), trace=trace
    )
    acc = np.zeros((BT, C), dtype=np.float32)
    for c in range(N_CORES):
        acc += res.results[c]["out"].astype(np.float32)
    out = acc + b_proj
    return out.reshape(B, T, C), res


def kernel(x, W_attn, b_attn, W_proj, b_proj):
    out, _ = kernel_with_results(x, W_attn, b_attn, W_proj, b_proj)
    return out


if __name__ == "__main__":
    import jax

    key = jax.random.key(0)
    ks = jax.random.split(key, 5)
    import jax.numpy as jnp

    inputs = {
        "x": jax.random.normal(ks[0], (B, T, C), dtype=jnp.float32),
        "W_attn": jax.random.normal(ks[1], (C, 3 * C), dtype=jnp.float32) * 0.02,
        "b_attn": jnp.zeros((3 * C,), dtype=jnp.float32),
        "W_proj": jax.random.normal(ks[2], (C, C), dtype=jnp.float32) * 0.02,
        "b_proj": jnp.zeros((C,), dtype=jnp.float32),
    }
    out = kernel(**{k: np.asarray(v) for k, v in inputs.items()})
    print(out.shape, out.dtype)
